# revision 51
# baseline (speedup 1.0000x reference)
"""Trainium2 Bass kernel for the two-template sparse cross-modal attention module.

Sharding: data-parallel over batch B=32 across 8 NeuronCores (4 samples/core).
Each sample carries two modality streams (v, i) that must be co-resident
because search tokens attend to the template keys of BOTH modalities.

Numerics: x and qkv_w ship as error-compensated fp8(e4m3) hi+residual pairs
(weights pre-scaled by 64); the QKV projections run as fp8 DoubleRow matmuls
at 2x the bf16 rate (Q/K with 2 compensation terms, V with 3 - score errors
average out through the softmax, V errors do not).  The 64x weight scale is
folded into the exp scale (scores carry 64^2) and the AV denominator column
(64), so no rescaling instructions exist anywhere.  Attention and the output
projection run in bf16; outputs stream back as bf16.

Per-core program (per sample s, streams st in {v, i}):
  A. QK^T in transposed layout (per-head Q.T, K.T as [64, tok] rows).
  B. V in natural layout [tok, 768] with a 64-valued denominator column per
     head ([tok, 65]) so the AV matmul also emits the softmax denominator.
  C. Attention per head: scores transposed (S.T[k, q] = K Q.T) in three
     single-bank PSUM tiles (separate tiles - dependency tracking is
     tile-granular), exp'd into bf16; AV in NATURAL layout
     (o[q, 65] = sum_k es[k, q].T-matmul v1[k, 65]) so the moving dim is 65
     instead of 384 and the denominator lands per-partition: normalization
     is one [128, TCH] reciprocal + one broadcast tensor_mul per head.
     Each q-chunk's PSUM accumulation group runs to completion before the
     next opens (start_tensor_calc lazily zeroes the whole tile).
  D. O natural -> O.T via PE transpose (identity matmul, [128,128] bf16).
  E. Transposed projection Y.T[cout, tok] = projw.T-chunks @ O.T with the
     bias folded into the PSUM->SBUF copy as a per-partition
     tensor_scalar_add.  Y.T DMAs out in bf16; the host transposes back.

Scheduling: one continuous software pipeline over all 96 heads.  Each step
emits the AV matmuls of the previous head (whose exps finished a full period
ago - they never wait on the Act engine), then the scores+exps of the
current head, with A/B work of the next sample and D/E work of the current
one spliced in between, paced by estimated PE time, so the PE runs at ~90%
occupancy end to end.  PSUM->SBUF copies split between DVE and Act (GPSIMD
cannot access PSUM).  Weight/input DMAs are sliced so the first matmul
starts ~3.5us in.
"""

import numpy as np

for _p in ("/opt/trn_rl_repo", "/root/.axon_site/_ro/trn_rl_repo"):
    import os
    import sys

    if os.path.isdir(_p) and _p not in sys.path:
        sys.path.append(_p)

B = 32
N_CORES = 8
SAMPLES = 4  # per core
C = 768
NTOK = 384
H = 12
DH = 64
MT = 128  # template tokens
CCH = C // 128  # 6 contraction chunks
MCH = 12  # QK row chunks (1536/128)
TCH = NTOK // 128  # 3 token chunks
WSCALE = 64.0
SCALE = DH ** (-0.5)
SSCALE = SCALE / (WSCALE * WSCALE)

_PROG_CACHE = {}


def _build_program():
    import concourse.bass as bass  # noqa: F401
    import concourse.tile as tile
    from concourse import bacc, mybir
    from concourse.masks import make_identity

    f32 = mybir.dt.float32
    bf16 = mybir.dt.bfloat16
    fp8 = mybir.dt.float8e4
    DR = mybir.MatmulPerfMode.DoubleRow
    Act = mybir.ActivationFunctionType

    nc = bacc.Bacc(None, target_bir_lowering=False)
    _lp = nc.allow_low_precision(reason="fp8/bf16 matmul inputs, fp32 PSUM accumulation")
    _lp.__enter__()

    # x and qkv_w ship as error-compensated fp8 pairs (hi + residual, both
    # e4m3, weights pre-scaled by WSCALE): three DoubleRow matmuls
    # (hi*hi + hi*lo + lo*hi) give bf16-grade accuracy at 2x the fp8 rate.
    # The 64x weight scale rides through scores (folded into the exp scale)
    # and through V (the denominator ones-column is 64 as well).
    xt_d = nc.dram_tensor(
        "xt", [2 * SAMPLES, 128, 2, CCH, NTOK], fp8, kind="ExternalInput"
    )
    # piece-major: piece j holds output-columns j*384..(j+1)*384, contiguous
    # per partition so each DMA descriptor is one 4.6KB run
    qkvw_d = nc.dram_tensor(
        "qkvwT", [CCH, 128, 2, CCH, 384], fp8, kind="ExternalInput"
    )
    projw_d = nc.dram_tensor("projwT", [128, CCH, C], bf16, kind="ExternalInput")
    bias_d = nc.dram_tensor("bias", [128, CCH], f32, kind="ExternalInput")
    y_d = nc.dram_tensor("y", [2 * SAMPLES, 128, CCH, NTOK], bf16, kind="ExternalOutput")

    with tile.TileContext(nc) as tc:
        with (
            tc.tile_pool(name="consts", bufs=1) as consts,
            tc.tile_pool(name="xtp", bufs=2) as xtp,
            tc.tile_pool(name="qktp", bufs=2) as qktp,
            tc.tile_pool(name="v1p", bufs=2) as v1p,
            tc.tile_pool(name="osbp", bufs=2) as osbp,
            tc.tile_pool(name="otTp", bufs=2) as otTp,
            tc.tile_pool(name="yp", bufs=2) as yp,
            tc.tile_pool(name="esp", bufs=4) as esp,
            tc.tile_pool(name="rp", bufs=4) as rp,
            tc.tile_pool(name="pap", bufs=3, space="PSUM") as pap,
            tc.tile_pool(name="pscap", bufs=1, space="PSUM") as pscap,
            tc.tile_pool(name="pscbp", bufs=1, space="PSUM") as pscbp,
            tc.tile_pool(name="psccp", bufs=1, space="PSUM") as psccp,
            tc.tile_pool(name="popp", bufs=2, space="PSUM") as popp,
        ):
            qkvw_sb = consts.tile([128, 2, CCH, 3 * C], fp8, name="qkvw")
            projw_sb = consts.tile([128, CCH, C], bf16)
            bias_sb = consts.tile([128, CCH], f32)
            ident = consts.tile([128, 128], bf16)
            make_identity(nc, ident)

            xt_t = [None] * SAMPLES
            qkt_t = [None] * SAMPLES
            v1_t = [None] * SAMPLES
            osb_t = [None] * SAMPLES
            otT_t = [None] * SAMPLES
            y_t = [[None, None] for _ in range(SAMPLES)]
            acopy_ctr = [0]  # round-robin counter for A-copy engine split

            def dma_const_units():
                # qkv weights split per 384-column piece and hi/lo half so
                # phase A can start after the first small transfers
                for j in range(CCH):
                    for p in range(2):
                        yield lambda j=j, p=p: nc.sync.dma_start(
                            out=qkvw_sb[:, p, :, j * 384 : (j + 1) * 384],
                            in_=qkvw_d[j, :, p],
                        )
                yield lambda: nc.sync.dma_start(out=projw_sb, in_=projw_d[:, :, :])
                yield lambda: nc.sync.dma_start(out=bias_sb, in_=bias_d[:, :])

            def dma_in_units(s):
                xt_t[s] = xtp.tile(
                    [128, 2, CCH, 2, NTOK], fp8, tag="xt", name=f"xt_{s}"
                )
                for st in range(2):
                    for p in range(2):
                        yield lambda st=st, p=p: nc.sync.dma_start(
                            out=xt_t[s][:, p, :, st, :],
                            in_=xt_d[2 * s + st, :, p],
                        )

            def psum_copy(out, in_):
                # GPSIMD cannot touch PSUM on hardware: split the PSUM->SBUF
                # copies between the Act engine (1 in 6) and the DVE
                if acopy_ctr[0] % 6 == 0:
                    nc.scalar.activation(out, in_, Act.Copy)
                else:
                    nc.vector.tensor_copy(out=out, in_=in_)
                acopy_ctr[0] += 1

            # compensated-fp8 DoubleRow contraction: hi*hi + hi*lo + lo*hi.
            # Q/K (phase A) drop the x-residual term (score errors average
            # out through the softmax; V errors do not), keeping 2 terms.
            HL = ((0, 0), (0, 1), (1, 0))
            HL_A = ((0, 0), (1, 0))

            def a_unit(s, st, m):
                pq = pap.tile([128, NTOK], f32, tag="pa", name=f"pa_a{s}_{st}_{m}")
                for wp, xp in HL_A:
                    for c2 in range(CCH // 2):
                        nc.tensor.matmul(
                            pq,
                            qkvw_sb[:, wp, 2 * c2 : 2 * c2 + 2, m * 128 : (m + 1) * 128],
                            xt_t[s][:, xp, 2 * c2 : 2 * c2 + 2, st, :],
                            start=((wp, xp) == HL_A[0] and c2 == 0),
                            stop=((wp, xp) == HL_A[-1] and c2 == CCH // 2 - 1),
                            perf_mode=DR,
                        )
                psum_copy(qkt_t[s][:, m, st, :], pq)

            def b_unit(s, st, t, n):
                pv = pap.tile([128, NTOK], f32, tag="pa", name=f"pa_b{s}_{st}_{t}_{n}")
                for xp, wp in HL:
                    for c2 in range(CCH // 2):
                        nc.tensor.matmul(
                            pv,
                            xt_t[s][
                                :, xp, 2 * c2 : 2 * c2 + 2, st, t * 128 : (t + 1) * 128
                            ],
                            qkvw_sb[
                                :, wp, 2 * c2 : 2 * c2 + 2,
                                2 * C + n * NTOK : 2 * C + (n + 1) * NTOK,
                            ],
                            start=((xp, wp) == HL[0] and c2 == 0),
                            stop=((xp, wp) == HL[-1] and c2 == CCH // 2 - 1),
                            perf_mode=DR,
                        )
                psum_copy(
                    v1_t[s][:, t, st, 6 * n : 6 * n + 6, 0:64],
                    pv.rearrange("p (h d) -> p h d", h=6),
                )
                if t == 0 and n == 0:
                    # 64 = WSCALE: the AV denominator column must carry the
                    # same scale as the (pre-scaled) V values
                    nc.vector.memset(v1_t[s][:, :, st, :, 64:65], 64.0)

            def ab_units(s):
                """Phase A (QK^T transposed) + phase B (V natural) for sample s,
                ordered so attention pair (st, hp) is ready as early as possible:
                A chunks hp-major (q then k, both streams), B chunks n-major."""
                qkt_t[s] = qktp.tile(
                    [128, MCH, 2, NTOK], bf16, tag="qkt", name=f"qkt_{s}"
                )
                v1_t[s] = v1p.tile(
                    [128, TCH, 2, H, 65], bf16, tag="v1", name=f"v1_{s}"
                )
                for st in range(2):
                    for m in range(MCH):
                        yield 480.0, (lambda st=st, m=m: a_unit(s, st, m))
                for st in range(2):
                    for n in range(2):
                        for t in range(TCH):
                            yield 720.0, (lambda st=st, t=t, n=n: b_unit(s, st, t, n))

            # Attention is emitted as a one-head software pipeline: the S
            # matmuls + exps of head h+1 are interleaved with the AV matmuls
            # of head h, so the PE->Act->PE loop of a single head never sits
            # on the critical path and the Act queue stays continuously fed.
            ht = {}  # (s, st, h) -> dict of live tiles

            def s_ab(s, st, h):
                """Scores for the own-mt (slot 0, exp A) and own-search
                (slot 1, exp B) keys, plus their exps."""
                qkt = qkt_t[s]
                # separate tiles per score slot: dependency tracking is
                # tile-granular, so a shared tile would serialize the next
                # head's score matmuls behind ALL of this head's exps
                pscA = pscap.tile([128, NTOK], f32, tag="pscA", name=f"pscA_{s}_{st}_{h}")
                pscB = pscbp.tile([128, 512], f32, tag="pscB", name=f"pscB_{s}_{st}_{h}")
                es = esp.tile([128, 3, 512], bf16, tag="es", name=f"es_{s}_{st}_{h}")
                ht[(s, st, h)] = {"pscA": pscA, "pscB": pscB, "es": es}
                ro = (h % 2) * 64
                hp = h // 2
                qT = qkt[ro : ro + 64, hp, st, :]
                kT = qkt[ro : ro + 64, 6 + hp, st, :]
                nc.tensor.matmul(pscA, kT[:, 0:MT], qT)
                nc.tensor.matmul(pscB[:, 0:256], kT[:, MT : MT + 128], qT[:, MT:])
                nc.tensor.matmul(pscB[:, 256:512], kT[:, MT + 128 :], qT[:, MT:])
                nc.scalar.activation(es[:, 0, 0:NTOK], pscA, Act.Exp, scale=SSCALE)
                nc.scalar.activation(es[:, 1, :], pscB, Act.Exp, scale=SSCALE)

            def s_c(s, st, h):
                """Scores for the other-modality template keys (slot 2) + exp."""
                qkt = qkt_t[s]
                t = ht[(s, st, h)]
                pscC = psccp.tile([128, 256], f32, tag="pscC", name=f"pscC_{s}_{st}_{h}")
                ro = (h % 2) * 64
                hp = h // 2
                qT = qkt[ro : ro + 64, hp, st, :]
                kTo = qkt[ro : ro + 64, 6 + hp, 1 - st, :]
                nc.tensor.matmul(pscC, kTo[:, 0:MT], qT[:, MT:])
                nc.scalar.activation(
                    t["es"][:, 2, 0:256], pscC, Act.Exp, scale=SSCALE
                )

            def av_full(s, st, h):
                """All AV matmuls for head h.  Runs a full pipeline period
                after the head's exps, so nothing here waits on the Act
                engine.  Each search q-chunk's accumulation group runs to
                completion before the next opens: start_tensor_calc lazily
                zeroes the whole PSUM tile, so interleaving open groups in
                one tile destroys the earlier group's partial sums.
                """
                v1 = v1_t[s]
                t = ht[(s, st, h)]
                es = t["es"]
                # 96-f32 stride keeps every matmul PSUM dst 16B-aligned
                po = popp.tile([128, TCH, 96], f32, tag="po", name=f"po_{s}_{st}_{h}")
                t["po"] = po
                # mt queries: attend own-mt keys only (closed group)
                nc.tensor.matmul(po[:, 0, 0:65], es[:, 0, 0:MT], v1[:, 0, st, h, :])
                for u in (1, 2):
                    qo = (u - 1) * 128
                    dst = po[:, u, 0:65]
                    nc.tensor.matmul(
                        dst, es[:, 0, MT + qo : MT + qo + 128],
                        v1[:, 0, st, h, :], start=True, stop=False,
                    )
                    nc.tensor.matmul(
                        dst, es[:, 1, qo : qo + 128],
                        v1[:, 1, st, h, :], start=False, stop=False,
                    )
                    nc.tensor.matmul(
                        dst, es[:, 1, 256 + qo : 256 + qo + 128],
                        v1[:, 2, st, h, :], start=False, stop=False,
                    )
                    nc.tensor.matmul(
                        dst, es[:, 2, qo : qo + 128],
                        v1[:, 0, 1 - st, h, :], start=False, stop=True,
                    )

            def av_finish(s, st, h):
                """Reciprocal of the denominator column + broadcast normalize."""
                if osb_t[s] is None:
                    osb_t[s] = osbp.tile(
                        [128, TCH, 2, C], bf16, tag="osb", name=f"osb_{s}"
                    )
                po = ht.pop((s, st, h))["po"]
                rl = rp.tile([128, TCH], f32, tag="rl", name=f"rl_{s}_{st}_{h}")
                nc.vector.reciprocal(out=rl, in_=po[:, :, 64:65])
                nc.vector.tensor_mul(
                    osb_t[s][:, :, st, h * 64 : (h + 1) * 64],
                    po[:, :, 0:64],
                    rl[:, :, None].broadcast_to([128, TCH, 64]),
                )

            def d_unit(s, st, cc):
                if otT_t[s] is None:
                    otT_t[s] = otTp.tile(
                        [128, CCH, 2, NTOK], bf16, tag="otT", name=f"otT_{s}"
                    )
                pt = pap.tile([128, NTOK], bf16, tag="pa", name=f"pa_d{s}_{st}_{cc}")
                for u in range(TCH):
                    nc.tensor.transpose(
                        pt[:, u * 128 : (u + 1) * 128],
                        osb_t[s][:, u, st, cc * 128 : (cc + 1) * 128],
                        ident,
                    )
                nc.vector.tensor_copy(out=otT_t[s][:, cc, st, :], in_=pt)

            def e_unit(s, st, m2):
                if y_t[s][st] is None:
                    y_t[s][st] = yp.tile(
                        [128, CCH, NTOK], bf16, tag="y", name=f"y_{s}_{st}"
                    )
                py = pap.tile([128, NTOK], f32, tag="pa", name=f"pa_e{s}_{st}_{m2}")
                for c in range(CCH):
                    nc.tensor.matmul(
                        py,
                        projw_sb[:, c, m2 * 128 : (m2 + 1) * 128],
                        otT_t[s][:, c, st, :],
                        start=(c == 0),
                        stop=(c == CCH - 1),
                    )
                nc.vector.tensor_scalar_add(
                    y_t[s][st][:, m2, :], py, bias_sb[:, m2 : m2 + 1]
                )
                # two half-DMAs per stream so the last one drains faster
                if m2 == CCH // 2 - 1 or m2 == CCH - 1:
                    half = m2 // (CCH // 2)
                    sl = slice(half * (CCH // 2), (half + 1) * (CCH // 2))
                    nc.sync.dma_start(
                        out=y_d[2 * s + st, :, sl, :], in_=y_t[s][st][:, sl, :]
                    )

            def de_units(s):
                for st in range(2):
                    for cc in range(CCH):
                        yield lambda st=st, cc=cc: d_unit(s, st, cc)
                    for m2 in range(CCH):
                        yield lambda st=st, m2=m2: e_unit(s, st, m2)

            # ---- software-pipelined emission ----
            # One continuous stream: attention heads of every sample in
            # sequence, with a single global filler queue (phases A/B of the
            # next sample, D/E of the current one as their inputs retire).
            # Filler is consumed at splice points inside each head, paced by
            # estimated PE time, and spills across sample boundaries.
            from collections import deque

            fill_q = deque()
            spent = [0.0]  # estimated PE-ns of filler consumed

            def splice_upto(tgt_ns):
                while fill_q and spent[0] < tgt_ns:
                    run_one()

            consts_dma = list(dma_const_units())
            first_in = list(dma_in_units(0))
            # DMA order: stream-0 x and its first weight pieces leapfrog so
            # the first matmuls start ~3.5us in; stream-1 x follows while
            # stream-0's phase A computes
            order = [first_in[0], consts_dma[0], first_in[1], consts_dma[1],
                     consts_dma[2], consts_dma[3], consts_dma[4],
                     consts_dma[5], consts_dma[6], consts_dma[7],
                     first_in[2], first_in[3]]
            for u in order:
                u()
            for u in consts_dma[8:]:
                u()
            for _, u in ab_units(0):
                u()

            # filler pacing: per-head PE-ns of filler, slightly below the
            # production rate so a backlog accumulates for the last sample
            PER_HEAD = 1150.0
            tgt_base = [0.0]
            appended = [0]  # items ever appended to fill_q
            ran = [0]  # items ever consumed

            def run_one():
                cost, u, then = fill_q.popleft()
                u()
                ran[0] += 1
                spent[0] += cost
                if then:
                    for item in then:
                        fill_q.append(item)
                        appended[0] += 1

            def push(cost, u, then=None):
                fill_q.append((cost, u, then))
                appended[0] += 1

            def after_finish(s, st, h):
                if h % 2 == 1:
                    hp = h // 2
                    then = None
                    if hp == 5:
                        then = [
                            (960.0,
                             (lambda s=s, st=st, m2=m2: e_unit(s, st, m2)),
                             None)
                            for m2 in range(CCH)
                        ]
                    push(200.0, (lambda s=s, st=st, hp=hp: d_unit(s, st, hp)),
                         then)

            markers = {}
            flat = [
                (s, st, h)
                for s in range(SAMPLES)
                for st in range(2)
                for h in range(H)
            ]
            per_head = [PER_HEAD]
            prev = [None]

            def step(cur):
                """One pipeline step: AV of the previous head wrapped around
                S+exp of the current one, filler spliced at the two points
                where the PE would otherwise wait on the Act engine."""
                s, st, h = cur
                if st == 0 and h == 0:
                    if s + 1 < SAMPLES:
                        for u in dma_in_units(s + 1):
                            push(0.0, u)
                        for cost, u in ab_units(s + 1):
                            push(cost, u)
                        # A/B of s+1 must be fully emitted before the first
                        # head of s+1 (the in-order PE queue would otherwise
                        # invert the qkt/v1 dependencies)
                        markers[s + 1] = appended[0]
                    else:
                        # final sample: spread the backlog + its own D/E
                        # evenly over the remaining heads
                        left = sum(c for c, _, _ in fill_q) + 2 * H * 600.0
                        per_head[0] = left / (2 * H)
                    if s > 0:
                        while ran[0] < markers[s]:
                            run_one()
                p = prev[0]
                if p is not None:
                    # AV of the previous head: all three exps it needs
                    # completed during the previous period, so none of these
                    # matmuls ever wait on the Act engine
                    av_full(*p)
                s_ab(s, st, h)
                s_c(s, st, h)
                splice_upto(tgt_base[0] + 0.8 * per_head[0])
                if p is not None:
                    av_finish(*p)
                    after_finish(*p)
                splice_upto(tgt_base[0] + per_head[0])
                tgt_base[0] += per_head[0]
                prev[0] = cur

            for cur in flat:
                step(cur)
            av_full(*prev[0])
            av_finish(*prev[0])
            after_finish(*prev[0])
            while fill_q:
                run_one()

    _lp.__exit__(None, None, None)
    nc.compile()
    return nc


def _get_program():
    if "prog" not in _PROG_CACHE:
        _PROG_CACHE["prog"] = _build_program()
    return _PROG_CACHE["prog"]


def _to_bf16(a):
    import ml_dtypes

    return np.ascontiguousarray(a.astype(ml_dtypes.bfloat16))


def _to_fp8_pair(a):
    import ml_dtypes

    f8 = ml_dtypes.float8_e4m3
    hi = a.astype(f8)
    lo = (a - hi.astype(np.float32)).astype(f8)
    return np.ascontiguousarray(hi), np.ascontiguousarray(lo)


def _prep_in_maps(x_v, x_i, qkv_w, proj_w, proj_b):
    # weights: [out, in] -> transposed [in, out] -> [128, CCH, out] chunked
    qkvwT = np.asarray(qkv_w, np.float32).T.reshape(CCH, 128, 3 * C).transpose(1, 0, 2)
    projwT = np.asarray(proj_w, np.float32).T.reshape(CCH, 128, C).transpose(1, 0, 2)
    bias = np.ascontiguousarray(
        np.asarray(proj_b, np.float32).reshape(CCH, 128).T
    )
    qkvwTh, qkvwTl = _to_fp8_pair(qkvwT * WSCALE)
    qkvwT8 = np.stack([qkvwTh, qkvwTl], axis=1)  # [128, 2, CCH, 3C]
    qkvwT8 = np.ascontiguousarray(
        qkvwT8.reshape(128, 2, CCH, CCH, 384).transpose(3, 0, 1, 2, 4)
    )
    projwT = _to_bf16(projwT)
    in_maps = []
    for core in range(N_CORES):
        sl = slice(core * SAMPLES, (core + 1) * SAMPLES)
        # streams interleaved: 2s = v-sample, 2s+1 = i-sample;
        # layout [128, CCH, NTOK]: partition p, chunk c -> channel c*128+p
        xs = np.empty((2 * SAMPLES, 128, CCH, NTOK), np.float32)
        xs[0::2] = (
            np.asarray(x_v[sl], np.float32)
            .transpose(0, 2, 1)
            .reshape(SAMPLES, CCH, 128, NTOK)
            .transpose(0, 2, 1, 3)
        )
        xs[1::2] = (
            np.asarray(x_i[sl], np.float32)
            .transpose(0, 2, 1)
            .reshape(SAMPLES, CCH, 128, NTOK)
            .transpose(0, 2, 1, 3)
        )
        xth, xtl = _to_fp8_pair(xs)
        xt8 = np.ascontiguousarray(np.stack([xth, xtl], axis=2))
        in_maps.append(
            {
                "xt": xt8,
                "qkvwT": qkvwT8,
                "projwT": projwT,
                "bias": bias,
            }
        )
    return in_maps


def _decode_out(res):
    out_v = np.empty((B, NTOK, C), np.float32)
    out_i = np.empty((B, NTOK, C), np.float32)
    for core in range(N_CORES):
        y = np.asarray(res.results[core]["y"], dtype=np.float32)
        # [2S, 128, CCH, NTOK] -> [2S, CCH*128 = C, NTOK] -> [2S, NTOK, C]
        y = y.transpose(0, 2, 1, 3).reshape(2 * SAMPLES, C, NTOK).transpose(0, 2, 1)
        sl = slice(core * SAMPLES, (core + 1) * SAMPLES)
        out_v[sl] = y[0::2]
        out_i[sl] = y[1::2]
    return out_v, out_i


def kernel(x_v, x_i, qkv_w, proj_w, proj_b, t_h, t_w, s_h, s_w, num_heads):
    from concourse.bass_utils import run_bass_kernel_spmd

    nc = _get_program()
    in_maps = _prep_in_maps(x_v, x_i, qkv_w, proj_w, proj_b)
    res = run_bass_kernel_spmd(nc, in_maps, list(range(N_CORES)))
    return _decode_out(res)


# revision 52
# speedup vs baseline: 1.0040x; 1.0040x over previous
"""Trainium2 Bass kernel for the two-template sparse cross-modal attention module.

Sharding: data-parallel over batch B=32 across 8 NeuronCores (4 samples/core).
Each sample carries two modality streams (v, i) that must be co-resident
because search tokens attend to the template keys of BOTH modalities.

Numerics: x and qkv_w ship as error-compensated fp8(e4m3) hi+residual pairs
(weights pre-scaled by 64); the QKV projections run as fp8 DoubleRow matmuls
at 2x the bf16 rate (Q/K with 2 compensation terms, V with 3 - score errors
average out through the softmax, V errors do not).  The 64x weight scale is
folded into the exp scale (scores carry 64^2) and the AV denominator column
(64), so no rescaling instructions exist anywhere.  Attention and the output
projection run in bf16; outputs stream back as bf16.

Per-core program (per sample s, streams st in {v, i}):
  A. QK^T in transposed layout (per-head Q.T, K.T as [64, tok] rows).
  B. V in natural layout [tok, 768] with a 64-valued denominator column per
     head ([tok, 65]) so the AV matmul also emits the softmax denominator.
  C. Attention per head: scores transposed (S.T[k, q] = K Q.T) in three
     single-bank PSUM tiles (separate tiles - dependency tracking is
     tile-granular), exp'd into bf16; AV in NATURAL layout
     (o[q, 65] = sum_k es[k, q].T-matmul v1[k, 65]) so the moving dim is 65
     instead of 384 and the denominator lands per-partition: normalization
     is one [128, TCH] reciprocal + one broadcast tensor_mul per head.
     Each q-chunk's PSUM accumulation group runs to completion before the
     next opens (start_tensor_calc lazily zeroes the whole tile).
  D. O natural -> O.T via PE transpose (identity matmul, [128,128] bf16).
  E. Transposed projection Y.T[cout, tok] = projw.T-chunks @ O.T with the
     bias folded into the PSUM->SBUF copy as a per-partition
     tensor_scalar_add.  Y.T DMAs out in bf16; the host transposes back.

Scheduling: one continuous software pipeline over all 96 heads.  Each step
emits the AV matmuls of the previous head (whose exps finished a full period
ago - they never wait on the Act engine), then the scores+exps of the
current head, with A/B work of the next sample and D/E work of the current
one spliced in between, paced by estimated PE time, so the PE runs at ~90%
occupancy end to end.  PSUM->SBUF copies split between DVE and Act (GPSIMD
cannot access PSUM).  Weight/input DMAs are sliced so the first matmul
starts ~3.5us in.
"""

import numpy as np

for _p in ("/opt/trn_rl_repo", "/root/.axon_site/_ro/trn_rl_repo"):
    import os
    import sys

    if os.path.isdir(_p) and _p not in sys.path:
        sys.path.append(_p)

B = 32
N_CORES = 8
SAMPLES = 4  # per core
C = 768
NTOK = 384
H = 12
DH = 64
MT = 128  # template tokens
CCH = C // 128  # 6 contraction chunks
MCH = 12  # QK row chunks (1536/128)
TCH = NTOK // 128  # 3 token chunks
WSCALE = 64.0
SCALE = DH ** (-0.5)
SSCALE = SCALE / (WSCALE * WSCALE)

_PROG_CACHE = {}


def _build_program():
    import concourse.bass as bass  # noqa: F401
    import concourse.tile as tile
    from concourse import bacc, mybir
    from concourse.masks import make_identity

    f32 = mybir.dt.float32
    bf16 = mybir.dt.bfloat16
    fp8 = mybir.dt.float8e4
    DR = mybir.MatmulPerfMode.DoubleRow
    Act = mybir.ActivationFunctionType

    nc = bacc.Bacc(None, target_bir_lowering=False)
    _lp = nc.allow_low_precision(reason="fp8/bf16 matmul inputs, fp32 PSUM accumulation")
    _lp.__enter__()

    # x and qkv_w ship as error-compensated fp8 pairs (hi + residual, both
    # e4m3, weights pre-scaled by WSCALE): three DoubleRow matmuls
    # (hi*hi + hi*lo + lo*hi) give bf16-grade accuracy at 2x the fp8 rate.
    # The 64x weight scale rides through scores (folded into the exp scale)
    # and through V (the denominator ones-column is 64 as well).
    xt_d = nc.dram_tensor(
        "xt", [2 * SAMPLES, 128, 2, CCH, NTOK], fp8, kind="ExternalInput"
    )
    # piece-major: piece j holds output-columns j*384..(j+1)*384, contiguous
    # per partition so each DMA descriptor is one 4.6KB run
    qkvw_d = nc.dram_tensor(
        "qkvwT", [CCH, 128, 2, CCH, 384], fp8, kind="ExternalInput"
    )
    projw_d = nc.dram_tensor("projwT", [128, CCH, C], bf16, kind="ExternalInput")
    bias_d = nc.dram_tensor("bias", [128, CCH], f32, kind="ExternalInput")
    y_d = nc.dram_tensor("y", [2 * SAMPLES, 128, CCH, NTOK], bf16, kind="ExternalOutput")

    with tile.TileContext(nc) as tc:
        with (
            tc.tile_pool(name="consts", bufs=1) as consts,
            tc.tile_pool(name="xtp", bufs=2) as xtp,
            tc.tile_pool(name="qktp", bufs=2) as qktp,
            tc.tile_pool(name="v1p", bufs=2) as v1p,
            tc.tile_pool(name="osbp", bufs=2) as osbp,
            tc.tile_pool(name="otTp", bufs=2) as otTp,
            tc.tile_pool(name="yp", bufs=2) as yp,
            tc.tile_pool(name="esp", bufs=4) as esp,
            tc.tile_pool(name="rp", bufs=4) as rp,
            tc.tile_pool(name="pap", bufs=3, space="PSUM") as pap,
            tc.tile_pool(name="pscap", bufs=1, space="PSUM") as pscap,
            tc.tile_pool(name="pscbp", bufs=1, space="PSUM") as pscbp,
            tc.tile_pool(name="psccp", bufs=1, space="PSUM") as psccp,
            tc.tile_pool(name="popp", bufs=2, space="PSUM") as popp,
        ):
            qkvw_sb = consts.tile([128, 2, CCH, 3 * C], fp8, name="qkvw")
            projw_sb = consts.tile([128, CCH, C], bf16)
            bias_sb = consts.tile([128, CCH], f32)
            ident = consts.tile([128, 128], bf16)
            make_identity(nc, ident)

            xt_t = [None] * SAMPLES
            qkt_t = [None] * SAMPLES
            v1_t = [None] * SAMPLES
            osb_t = [None] * SAMPLES
            otT_t = [None] * SAMPLES
            y_t = [[None, None] for _ in range(SAMPLES)]
            acopy_ctr = [0]  # round-robin counter for A-copy engine split

            def dma_const_units():
                # qkv weights split per 384-column piece and hi/lo half so
                # phase A can start after the first small transfers
                for j in range(CCH):
                    for p in range(2):
                        yield lambda j=j, p=p: nc.sync.dma_start(
                            out=qkvw_sb[:, p, :, j * 384 : (j + 1) * 384],
                            in_=qkvw_d[j, :, p],
                        )
                yield lambda: nc.sync.dma_start(out=projw_sb, in_=projw_d[:, :, :])
                yield lambda: nc.sync.dma_start(out=bias_sb, in_=bias_d[:, :])

            def dma_in_units(s):
                xt_t[s] = xtp.tile(
                    [128, 2, CCH, 2, NTOK], fp8, tag="xt", name=f"xt_{s}"
                )
                for st in range(2):
                    for p in range(2):
                        yield lambda st=st, p=p: nc.sync.dma_start(
                            out=xt_t[s][:, p, :, st, :],
                            in_=xt_d[2 * s + st, :, p],
                        )

            def psum_copy(out, in_):
                # GPSIMD cannot touch PSUM on hardware: split the PSUM->SBUF
                # copies between the Act engine (1 in 6) and the DVE
                if acopy_ctr[0] % 6 == 0:
                    nc.scalar.activation(out, in_, Act.Copy)
                else:
                    nc.vector.tensor_copy(out=out, in_=in_)
                acopy_ctr[0] += 1

            # compensated-fp8 DoubleRow contraction: hi*hi + hi*lo + lo*hi.
            # Q/K (phase A) drop the x-residual term (score errors average
            # out through the softmax; V errors do not), keeping 2 terms.
            HL = ((0, 0), (0, 1), (1, 0))
            HL_A = ((0, 0), (1, 0))

            def a_unit(s, st, m):
                pq = pap.tile([128, NTOK], f32, tag="pa", name=f"pa_a{s}_{st}_{m}")
                for wp, xp in HL_A:
                    for c2 in range(CCH // 2):
                        nc.tensor.matmul(
                            pq,
                            qkvw_sb[:, wp, 2 * c2 : 2 * c2 + 2, m * 128 : (m + 1) * 128],
                            xt_t[s][:, xp, 2 * c2 : 2 * c2 + 2, st, :],
                            start=((wp, xp) == HL_A[0] and c2 == 0),
                            stop=((wp, xp) == HL_A[-1] and c2 == CCH // 2 - 1),
                            perf_mode=DR,
                        )
                psum_copy(qkt_t[s][:, m, st, :], pq)

            def b_unit(s, st, t, n):
                pv = pap.tile([128, NTOK], f32, tag="pa", name=f"pa_b{s}_{st}_{t}_{n}")
                for xp, wp in HL:
                    for c2 in range(CCH // 2):
                        nc.tensor.matmul(
                            pv,
                            xt_t[s][
                                :, xp, 2 * c2 : 2 * c2 + 2, st, t * 128 : (t + 1) * 128
                            ],
                            qkvw_sb[
                                :, wp, 2 * c2 : 2 * c2 + 2,
                                2 * C + n * NTOK : 2 * C + (n + 1) * NTOK,
                            ],
                            start=((xp, wp) == HL[0] and c2 == 0),
                            stop=((xp, wp) == HL[-1] and c2 == CCH // 2 - 1),
                            perf_mode=DR,
                        )
                psum_copy(
                    v1_t[s][:, t, st, 6 * n : 6 * n + 6, 0:64],
                    pv.rearrange("p (h d) -> p h d", h=6),
                )
                if t == 0 and n == 0:
                    # 64 = WSCALE: the AV denominator column must carry the
                    # same scale as the (pre-scaled) V values
                    nc.vector.memset(v1_t[s][:, :, st, :, 64:65], 64.0)

            def ab_units(s):
                """Phase A (QK^T transposed) + phase B (V natural) for sample s,
                ordered so attention pair (st, hp) is ready as early as possible:
                A chunks hp-major (q then k, both streams), B chunks n-major."""
                qkt_t[s] = qktp.tile(
                    [128, MCH, 2, NTOK], bf16, tag="qkt", name=f"qkt_{s}"
                )
                v1_t[s] = v1p.tile(
                    [128, TCH, 2, H, 65], bf16, tag="v1", name=f"v1_{s}"
                )
                for st in range(2):
                    for m in range(MCH):
                        yield 480.0, (lambda st=st, m=m: a_unit(s, st, m))
                for st in range(2):
                    for n in range(2):
                        for t in range(TCH):
                            yield 720.0, (lambda st=st, t=t, n=n: b_unit(s, st, t, n))

            # Attention is emitted as a one-head software pipeline: the S
            # matmuls + exps of head h+1 are interleaved with the AV matmuls
            # of head h, so the PE->Act->PE loop of a single head never sits
            # on the critical path and the Act queue stays continuously fed.
            ht = {}  # (s, st, h) -> dict of live tiles

            def s_ab(s, st, h):
                """Scores for the own-mt (slot 0, exp A) and own-search
                (slot 1, exp B) keys, plus their exps."""
                qkt = qkt_t[s]
                # separate tiles per score slot: dependency tracking is
                # tile-granular, so a shared tile would serialize the next
                # head's score matmuls behind ALL of this head's exps
                pscA = pscap.tile([128, NTOK], f32, tag="pscA", name=f"pscA_{s}_{st}_{h}")
                pscB = pscbp.tile([128, 512], f32, tag="pscB", name=f"pscB_{s}_{st}_{h}")
                es = esp.tile([128, 3, 512], bf16, tag="es", name=f"es_{s}_{st}_{h}")
                ht[(s, st, h)] = {"pscA": pscA, "pscB": pscB, "es": es}
                ro = (h % 2) * 64
                hp = h // 2
                qT = qkt[ro : ro + 64, hp, st, :]
                kT = qkt[ro : ro + 64, 6 + hp, st, :]
                nc.tensor.matmul(pscA, kT[:, 0:MT], qT)
                nc.tensor.matmul(pscB[:, 0:256], kT[:, MT : MT + 128], qT[:, MT:])
                nc.tensor.matmul(pscB[:, 256:512], kT[:, MT + 128 :], qT[:, MT:])
                nc.scalar.activation(es[:, 0, 0:NTOK], pscA, Act.Exp, scale=SSCALE)
                nc.scalar.activation(es[:, 1, :], pscB, Act.Exp, scale=SSCALE)

            def s_c(s, st, h):
                """Scores for the other-modality template keys (slot 2) + exp."""
                qkt = qkt_t[s]
                t = ht[(s, st, h)]
                pscC = psccp.tile([128, 256], f32, tag="pscC", name=f"pscC_{s}_{st}_{h}")
                ro = (h % 2) * 64
                hp = h // 2
                qT = qkt[ro : ro + 64, hp, st, :]
                kTo = qkt[ro : ro + 64, 6 + hp, 1 - st, :]
                nc.tensor.matmul(pscC, kTo[:, 0:MT], qT[:, MT:])
                nc.scalar.activation(
                    t["es"][:, 2, 0:256], pscC, Act.Exp, scale=SSCALE
                )

            def av_full(s, st, h):
                """All AV matmuls for head h.  Runs a full pipeline period
                after the head's exps, so nothing here waits on the Act
                engine.  Each search q-chunk's accumulation group runs to
                completion before the next opens: start_tensor_calc lazily
                zeroes the whole PSUM tile, so interleaving open groups in
                one tile destroys the earlier group's partial sums.
                """
                v1 = v1_t[s]
                t = ht[(s, st, h)]
                es = t["es"]
                # 96-f32 stride keeps every matmul PSUM dst 16B-aligned
                po = popp.tile([128, TCH, 96], f32, tag="po", name=f"po_{s}_{st}_{h}")
                t["po"] = po
                # mt queries: attend own-mt keys only (closed group)
                nc.tensor.matmul(po[:, 0, 0:65], es[:, 0, 0:MT], v1[:, 0, st, h, :])
                for u in (1, 2):
                    qo = (u - 1) * 128
                    dst = po[:, u, 0:65]
                    nc.tensor.matmul(
                        dst, es[:, 0, MT + qo : MT + qo + 128],
                        v1[:, 0, st, h, :], start=True, stop=False,
                    )
                    nc.tensor.matmul(
                        dst, es[:, 1, qo : qo + 128],
                        v1[:, 1, st, h, :], start=False, stop=False,
                    )
                    nc.tensor.matmul(
                        dst, es[:, 1, 256 + qo : 256 + qo + 128],
                        v1[:, 2, st, h, :], start=False, stop=False,
                    )
                    nc.tensor.matmul(
                        dst, es[:, 2, qo : qo + 128],
                        v1[:, 0, 1 - st, h, :], start=False, stop=True,
                    )

            def av_finish(s, st, h):
                """Reciprocal of the denominator column + broadcast normalize."""
                if osb_t[s] is None:
                    osb_t[s] = osbp.tile(
                        [128, TCH, 2, C], bf16, tag="osb", name=f"osb_{s}"
                    )
                po = ht.pop((s, st, h))["po"]
                rl = rp.tile([128, TCH], f32, tag="rl", name=f"rl_{s}_{st}_{h}")
                nc.vector.reciprocal(out=rl, in_=po[:, :, 64:65])
                nc.vector.tensor_mul(
                    osb_t[s][:, :, st, h * 64 : (h + 1) * 64],
                    po[:, :, 0:64],
                    rl[:, :, None].broadcast_to([128, TCH, 64]),
                )

            def d_unit(s, st, cc):
                if otT_t[s] is None:
                    otT_t[s] = otTp.tile(
                        [128, CCH, 2, NTOK], bf16, tag="otT", name=f"otT_{s}"
                    )
                pt = pap.tile([128, NTOK], bf16, tag="pa", name=f"pa_d{s}_{st}_{cc}")
                for u in range(TCH):
                    nc.tensor.transpose(
                        pt[:, u * 128 : (u + 1) * 128],
                        osb_t[s][:, u, st, cc * 128 : (cc + 1) * 128],
                        ident,
                    )
                nc.vector.tensor_copy(out=otT_t[s][:, cc, st, :], in_=pt)

            def e_unit(s, st, m2):
                if y_t[s][st] is None:
                    y_t[s][st] = yp.tile(
                        [128, CCH, NTOK], bf16, tag="y", name=f"y_{s}_{st}"
                    )
                py = pap.tile([128, NTOK], f32, tag="pa", name=f"pa_e{s}_{st}_{m2}")
                for c in range(CCH):
                    nc.tensor.matmul(
                        py,
                        projw_sb[:, c, m2 * 128 : (m2 + 1) * 128],
                        otT_t[s][:, c, st, :],
                        start=(c == 0),
                        stop=(c == CCH - 1),
                    )
                nc.vector.tensor_scalar_add(
                    y_t[s][st][:, m2, :], py, bias_sb[:, m2 : m2 + 1]
                )
                # two half-DMAs per stream so the last one drains faster
                if m2 == CCH // 2 - 1 or m2 == CCH - 1:
                    half = m2 // (CCH // 2)
                    sl = slice(half * (CCH // 2), (half + 1) * (CCH // 2))
                    nc.sync.dma_start(
                        out=y_d[2 * s + st, :, sl, :], in_=y_t[s][st][:, sl, :]
                    )

            def de_units(s):
                for st in range(2):
                    for cc in range(CCH):
                        yield lambda st=st, cc=cc: d_unit(s, st, cc)
                    for m2 in range(CCH):
                        yield lambda st=st, m2=m2: e_unit(s, st, m2)

            # ---- software-pipelined emission ----
            # One continuous stream: attention heads of every sample in
            # sequence, with a single global filler queue (phases A/B of the
            # next sample, D/E of the current one as their inputs retire).
            # Filler is consumed at splice points inside each head, paced by
            # estimated PE time, and spills across sample boundaries.
            from collections import deque

            fill_q = deque()
            spent = [0.0]  # estimated PE-ns of filler consumed

            def splice_upto(tgt_ns):
                while fill_q and spent[0] < tgt_ns:
                    run_one()

            consts_dma = list(dma_const_units())
            first_in = list(dma_in_units(0))
            # DMA order: stream-0 x and its first weight pieces leapfrog so
            # the first matmuls start ~3.5us in; stream-1 x follows while
            # stream-0's phase A computes
            order = [first_in[0], consts_dma[0], consts_dma[1],
                     consts_dma[2], consts_dma[3], consts_dma[4],
                     consts_dma[5], consts_dma[6], consts_dma[7],
                     first_in[2], first_in[1], first_in[3]]
            for u in order:
                u()
            for u in consts_dma[8:]:
                u()
            for _, u in ab_units(0):
                u()

            # filler pacing: per-head PE-ns of filler, slightly below the
            # production rate so a backlog accumulates for the last sample
            PER_HEAD = 1150.0
            tgt_base = [0.0]
            appended = [0]  # items ever appended to fill_q
            ran = [0]  # items ever consumed

            def run_one():
                cost, u, then = fill_q.popleft()
                u()
                ran[0] += 1
                spent[0] += cost
                if then:
                    for item in then:
                        fill_q.append(item)
                        appended[0] += 1

            def push(cost, u, then=None):
                fill_q.append((cost, u, then))
                appended[0] += 1

            def after_finish(s, st, h):
                if h % 2 == 1:
                    hp = h // 2
                    then = None
                    if hp == 5:
                        then = [
                            (960.0,
                             (lambda s=s, st=st, m2=m2: e_unit(s, st, m2)),
                             None)
                            for m2 in range(CCH)
                        ]
                    push(200.0, (lambda s=s, st=st, hp=hp: d_unit(s, st, hp)),
                         then)

            markers = {}
            flat = [
                (s, st, h)
                for s in range(SAMPLES)
                for st in range(2)
                for h in range(H)
            ]
            per_head = [PER_HEAD]
            prev = [None]

            def step(cur):
                """One pipeline step: AV of the previous head wrapped around
                S+exp of the current one, filler spliced at the two points
                where the PE would otherwise wait on the Act engine."""
                s, st, h = cur
                if st == 0 and h == 0:
                    if s + 1 < SAMPLES:
                        for u in dma_in_units(s + 1):
                            push(0.0, u)
                        for cost, u in ab_units(s + 1):
                            push(cost, u)
                        # A/B of s+1 must be fully emitted before the first
                        # head of s+1 (the in-order PE queue would otherwise
                        # invert the qkt/v1 dependencies)
                        markers[s + 1] = appended[0]
                    else:
                        # final sample: spread the backlog + its own D/E
                        # evenly over the remaining heads
                        left = sum(c for c, _, _ in fill_q) + 2 * H * 600.0
                        per_head[0] = left / (2 * H)
                    if s > 0:
                        while ran[0] < markers[s]:
                            run_one()
                p = prev[0]
                if p is not None:
                    # AV of the previous head: all three exps it needs
                    # completed during the previous period, so none of these
                    # matmuls ever wait on the Act engine
                    av_full(*p)
                s_ab(s, st, h)
                s_c(s, st, h)
                splice_upto(tgt_base[0] + 0.8 * per_head[0])
                if p is not None:
                    av_finish(*p)
                    after_finish(*p)
                splice_upto(tgt_base[0] + per_head[0])
                tgt_base[0] += per_head[0]
                prev[0] = cur

            for cur in flat:
                step(cur)
            av_full(*prev[0])
            av_finish(*prev[0])
            after_finish(*prev[0])
            while fill_q:
                run_one()

    _lp.__exit__(None, None, None)
    nc.compile()
    return nc


def _get_program():
    if "prog" not in _PROG_CACHE:
        _PROG_CACHE["prog"] = _build_program()
    return _PROG_CACHE["prog"]


def _to_bf16(a):
    import ml_dtypes

    return np.ascontiguousarray(a.astype(ml_dtypes.bfloat16))


def _to_fp8_pair(a):
    import ml_dtypes

    f8 = ml_dtypes.float8_e4m3
    hi = a.astype(f8)
    lo = (a - hi.astype(np.float32)).astype(f8)
    return np.ascontiguousarray(hi), np.ascontiguousarray(lo)


def _prep_in_maps(x_v, x_i, qkv_w, proj_w, proj_b):
    # weights: [out, in] -> transposed [in, out] -> [128, CCH, out] chunked
    qkvwT = np.asarray(qkv_w, np.float32).T.reshape(CCH, 128, 3 * C).transpose(1, 0, 2)
    projwT = np.asarray(proj_w, np.float32).T.reshape(CCH, 128, C).transpose(1, 0, 2)
    bias = np.ascontiguousarray(
        np.asarray(proj_b, np.float32).reshape(CCH, 128).T
    )
    qkvwTh, qkvwTl = _to_fp8_pair(qkvwT * WSCALE)
    qkvwT8 = np.stack([qkvwTh, qkvwTl], axis=1)  # [128, 2, CCH, 3C]
    qkvwT8 = np.ascontiguousarray(
        qkvwT8.reshape(128, 2, CCH, CCH, 384).transpose(3, 0, 1, 2, 4)
    )
    projwT = _to_bf16(projwT)
    in_maps = []
    for core in range(N_CORES):
        sl = slice(core * SAMPLES, (core + 1) * SAMPLES)
        # streams interleaved: 2s = v-sample, 2s+1 = i-sample;
        # layout [128, CCH, NTOK]: partition p, chunk c -> channel c*128+p
        xs = np.empty((2 * SAMPLES, 128, CCH, NTOK), np.float32)
        xs[0::2] = (
            np.asarray(x_v[sl], np.float32)
            .transpose(0, 2, 1)
            .reshape(SAMPLES, CCH, 128, NTOK)
            .transpose(0, 2, 1, 3)
        )
        xs[1::2] = (
            np.asarray(x_i[sl], np.float32)
            .transpose(0, 2, 1)
            .reshape(SAMPLES, CCH, 128, NTOK)
            .transpose(0, 2, 1, 3)
        )
        xth, xtl = _to_fp8_pair(xs)
        xt8 = np.ascontiguousarray(np.stack([xth, xtl], axis=2))
        in_maps.append(
            {
                "xt": xt8,
                "qkvwT": qkvwT8,
                "projwT": projwT,
                "bias": bias,
            }
        )
    return in_maps


def _decode_out(res):
    out_v = np.empty((B, NTOK, C), np.float32)
    out_i = np.empty((B, NTOK, C), np.float32)
    for core in range(N_CORES):
        y = np.asarray(res.results[core]["y"], dtype=np.float32)
        # [2S, 128, CCH, NTOK] -> [2S, CCH*128 = C, NTOK] -> [2S, NTOK, C]
        y = y.transpose(0, 2, 1, 3).reshape(2 * SAMPLES, C, NTOK).transpose(0, 2, 1)
        sl = slice(core * SAMPLES, (core + 1) * SAMPLES)
        out_v[sl] = y[0::2]
        out_i[sl] = y[1::2]
    return out_v, out_i


def kernel(x_v, x_i, qkv_w, proj_w, proj_b, t_h, t_w, s_h, s_w, num_heads):
    from concourse.bass_utils import run_bass_kernel_spmd

    nc = _get_program()
    in_maps = _prep_in_maps(x_v, x_i, qkv_w, proj_w, proj_b)
    res = run_bass_kernel_spmd(nc, in_maps, list(range(N_CORES)))
    return _decode_out(res)
